# revision 3
# baseline (speedup 1.0000x reference)
"""Trainium2 Bass kernel for CharacterNet segment-mean + FC (segment_reduce).

Reference computation (per batch row b of 32):
  x = all_encoder_layers[layer_index][b]          # (512, 768)
  for t in 0..255: mean_t = mean(x[token_map[b,t]:token_map[b,t+1]])
  ote[b*256+t] = mean_t                           # (8192, 768) output 2
  rep = ote @ fc_w.T + fc_b                       # (8192, 768) output 1

Strategy: data-parallel over batch across 8 NeuronCores (4 rows/core).
The segment mean is a matmul with a one-hot-per-row selection matrix
SelT (512, 256), SelT[s, t] = (seg(s)==t) / count(seg(s)), built on
device from two tiny per-position index vectors with a single
tensor_scalar op per (128,256) chunk.  Stage 1 computes meanT = x.T @
SelT (H on partitions) so stage 2 (the FC) can consume it directly as
the stationary operand; the natural-orientation ote output is produced
with PE transposes.  Matmuls run as float32r (TF32-precision, full PE
rate); everything else stays fp32.
"""

import os
import numpy as np

import concourse.bass as bass
import concourse.bacc as bacc
import concourse.mybir as mybir
import concourse.tile as tile
from concourse import masks
from concourse.bass_utils import run_bass_kernel_spmd

N_CORES = 8
B, S, H, T = 32, 512, 768, 256
B_LOC = B // N_CORES          # 4 batch rows per core
NS = S // 128                 # 4 s-chunks per row
NJ = B_LOC * NS               # 16 (128,768) x chunks per core
NH = H // 128                 # 6 h-chunks
NB2 = 384                     # stage-2 N tile (two per 768)

F32 = mybir.dt.float32
# float32r = TF32-precision matmul at full PE rate (1 cyc/row for N>=256)
# vs plain fp32 at 4 cyc/row.  Switchable for accuracy fallback.
MM_DT = mybir.dt.float32r if os.environ.get("KERNEL_MM_DT", "f32r") == "f32r" else F32


def _f32(ap):
    # view an MM_DT tile as plain fp32 (exact datapath, e.g. PE transpose)
    return ap.bitcast(F32) if MM_DT != F32 else ap


def _r(ap):
    # view a fp32 DRAM region as MM_DT for a byte-copy DMA into an MM_DT tile
    return ap.bitcast(MM_DT) if MM_DT != F32 else ap


def build_kernel(reps: int = 1) -> bass.Bass:
    nc = bacc.Bacc("TRN2", target_bir_lowering=False, debug=False,
                   num_devices=N_CORES)

    x_d = nc.dram_tensor("x", (NJ * 128, H), F32, kind="ExternalInput")
    seg_d = nc.dram_tensor("selseg", (128, NJ), F32, kind="ExternalInput")
    inv_d = nc.dram_tensor("selinv", (128, NJ), F32, kind="ExternalInput")
    fcw_d = nc.dram_tensor("fcwT", (H, H), F32, kind="ExternalInput")
    fcb_d = nc.dram_tensor("fcb", (1, H), F32, kind="ExternalInput")
    ones_d = nc.dram_tensor("ones", (1, 128), F32, kind="ExternalInput")
    rep_d = nc.dram_tensor("rep", (B_LOC * T, H), F32, kind="ExternalOutput")
    ote_d = nc.dram_tensor("ote", (B_LOC * T, H), F32, kind="ExternalOutput")

    with tile.TileContext(nc) as tc:
        with (
            tc.tile_pool(name="const", bufs=1) as cpool,
            tc.tile_pool(name="xp", bufs=1) as xpool,
            tc.tile_pool(name="selp", bufs=1) as selpool,
            tc.tile_pool(name="mp", bufs=1) as mpool,
            tc.tile_pool(name="wp", bufs=1) as wpool,
            tc.tile_pool(name="ob", bufs=4) as opool,
            tc.tile_pool(name="p1", bufs=3, space="PSUM") as p1pool,
            tc.tile_pool(name="pt", bufs=2, space="PSUM") as ptpool,
            tc.tile_pool(name="p2", bufs=3, space="PSUM") as p2pool,
        ):
            # one-time constants
            iota_t = cpool.tile([128, T], F32, tag="iota")
            nc.gpsimd.iota(iota_t[:], pattern=[[1, T]], base=0,
                           channel_multiplier=0,
                           allow_small_or_imprecise_dtypes=True)
            ident = cpool.tile([128, 128], F32, tag="ident")
            masks.make_identity(nc, ident[:])
            ones = cpool.tile([1, 128], MM_DT, tag="ones")
            nc.sync.dma_start(ones[:], _r(ones_d[:]))

            for _ in range(reps):
                seg_sb = cpool.tile([128, NJ], F32, tag="seg")
                inv_sb = cpool.tile([128, NJ], F32, tag="inv")
                fcb_sb = cpool.tile([1, H], MM_DT, tag="fcb")
                nc.sync.dma_start(seg_sb[:], seg_d[:])
                nc.sync.dma_start(inv_sb[:], inv_d[:])
                nc.sync.dma_start(fcb_sb[:], _r(fcb_d[:]))

                w_sb = []
                for k in range(NH):
                    w = wpool.tile([128, H], MM_DT, tag=f"w{k}")
                    nc.sync.dma_start(w[:], _r(fcw_d[k * 128:(k + 1) * 128, :]))
                    w_sb.append(w)

                x_sb, sel_sb = [], []
                for j in range(NJ):
                    x = xpool.tile([128, H], MM_DT, tag=f"x{j}")
                    nc.sync.dma_start(x[:], _r(x_d[j * 128:(j + 1) * 128, :]))
                    x_sb.append(x)
                    sel = selpool.tile([128, T], MM_DT, tag=f"s{j}")
                    # Sel^T chunk: (s==seg member of segment t) * 1/count
                    nc.vector.tensor_scalar(
                        sel[:], iota_t[:],
                        seg_sb[:, j:j + 1], inv_sb[:, j:j + 1],
                        op0=mybir.AluOpType.is_equal,
                        op1=mybir.AluOpType.mult)
                    sel_sb.append(sel)

                for b in range(B_LOC):
                    # stage 1: meanT[b] (768, 256) = x[b].T @ SelT[b]
                    mb = []
                    for mh in range(NH):
                        m = mpool.tile([128, T], MM_DT, tag=f"m{b}_{mh}")
                        ps = p1pool.tile([128, T], F32, tag="ps1")
                        for ks in range(NS):
                            j = b * NS + ks
                            nc.tensor.matmul(
                                ps[:],
                                x_sb[j][:, mh * 128:(mh + 1) * 128],
                                sel_sb[j][:],
                                start=(ks == 0), stop=(ks == NS - 1))
                        nc.vector.tensor_copy(m[:], ps[:])
                        mb.append(m)

                    # ote rows of b: transpose meanT chunks to natural layout
                    for tq in range(2):
                        osb = opool.tile([128, H], F32, tag="osb")
                        for mh in range(NH):
                            pt = ptpool.tile([128, 128], F32, tag="pst")
                            nc.tensor.transpose(
                                pt[:], _f32(mb[mh][:, tq * 128:(tq + 1) * 128]),
                                ident[:])
                            nc.vector.tensor_copy(
                                osb[:, mh * 128:(mh + 1) * 128], pt[:])
                        r0 = (b * 2 + tq) * 128
                        nc.sync.dma_start(ote_d[r0:r0 + 128, :], osb[:])

                    # stage 2: rep rows of b = meanT.T @ fc_w.T + fc_b
                    for tq in range(2):
                        rsb = opool.tile([128, H], F32, tag="rsb")
                        for nh in range(2):
                            ps2 = p2pool.tile([128, NB2], F32, tag="ps2")
                            nsl = slice(nh * NB2, (nh + 1) * NB2)
                            for kh in range(NH):
                                nc.tensor.matmul(
                                    ps2[:],
                                    mb[kh][:, tq * 128:(tq + 1) * 128],
                                    w_sb[kh][:, nsl],
                                    start=(kh == 0), stop=False)
                            nc.tensor.matmul(
                                ps2[:], ones[:1, :], fcb_sb[:1, nsl],
                                start=False, stop=True)
                            nc.scalar.copy(rsb[:, nsl], ps2[:])
                        r0 = (b * 2 + tq) * 128
                        nc.sync.dma_start(rep_d[r0:r0 + 128, :], rsb[:])

    nc.compile()
    return nc


def _host_prep(all_encoder_layers, token_map, fc_w, fc_b, layer_index):
    """Slice the chosen layer and build per-core input maps."""
    layer = int(np.asarray(layer_index))
    x_full = np.ascontiguousarray(
        np.asarray(all_encoder_layers)[layer], dtype=np.float32)  # (B, S, H)
    tm = np.asarray(token_map).astype(np.int64)                   # (B, T+1)

    pos = np.arange(S)
    seg = np.empty((B, S), dtype=np.int64)
    for b in range(B):
        seg[b] = np.searchsorted(tm[b], pos, side="right") - 1
    valid = pos[None, :] < tm[:, -1:]
    seg = np.where(valid, np.clip(seg, 0, T - 1), T)              # (B, S)
    counts = (tm[:, 1:] - tm[:, :-1]).astype(np.float32)          # (B, T)
    inv = np.zeros((B, S), dtype=np.float32)
    bb = np.arange(B)[:, None]
    iv = seg < T
    inv[iv] = (np.float32(1.0) /
               counts[np.broadcast_to(bb, seg.shape)[iv], seg[iv]])

    fcwT = np.ascontiguousarray(np.asarray(fc_w, dtype=np.float32).T)
    fcb = np.asarray(fc_b, dtype=np.float32).reshape(1, H)

    in_maps = []
    for c in range(N_CORES):
        bs = slice(c * B_LOC, (c + 1) * B_LOC)
        # (B_LOC, S) -> (128, NJ) with column j = b*NS + chunk
        seg_t = np.ascontiguousarray(
            seg[bs].reshape(NJ, 128).T.astype(np.float32))
        inv_t = np.ascontiguousarray(inv[bs].reshape(NJ, 128).T)
        in_maps.append({
            "x": np.ascontiguousarray(x_full[bs].reshape(NJ * 128, H)),
            "selseg": seg_t,
            "selinv": inv_t,
            "fcwT": fcwT,
            "fcb": fcb,
            "ones": np.ones((1, 128), dtype=np.float32),
        })
    return in_maps


_NC_CACHE: dict = {}


def get_nc(reps: int = 1):
    if reps not in _NC_CACHE:
        _NC_CACHE[reps] = build_kernel(reps)
    return _NC_CACHE[reps]


def run_on_hw(in_maps, reps: int = 1):
    nc = get_nc(reps)
    return run_bass_kernel_spmd(nc, in_maps, list(range(N_CORES)), trace=False)


def kernel(all_encoder_layers, input_mask, token_map, fc_w, fc_b, layer_index):
    in_maps = _host_prep(all_encoder_layers, token_map, fc_w, fc_b, layer_index)
    res = run_on_hw(in_maps)
    rep = np.concatenate([res.results[c]["rep"] for c in range(N_CORES)], 0)
    ote = np.concatenate([res.results[c]["ote"] for c in range(N_CORES)], 0)
    return rep.astype(np.float32), ote.astype(np.float32)


# revision 15
# speedup vs baseline: 597.2192x; 597.2192x over previous
"""Trainium2 Bass kernel for CharacterNet segment-mean + FC (segment_reduce).

Reference computation (per batch row b of 32):
  x = all_encoder_layers[layer_index][b]          # (512, 768)
  for t in 0..255: mean_t = mean(x[token_map[b,t]:token_map[b,t+1]])
  ote[b*256+t] = mean_t                           # (8192, 768) output 2
  rep = ote @ fc_w.T + fc_b                       # (8192, 768) output 1

Strategy: data-parallel over batch across 8 NeuronCores (4 rows/core).
The segment mean is a matmul with a one-hot-per-row selection matrix
SelT (512, 256), SelT[s, t] = (seg(s)==t) / count(seg(s)), built on
device from two tiny per-position index vectors with a single
tensor_scalar op per (128,256) chunk.  Stage 1 computes meanT = x.T @
SelT (H on partitions) so stage 2 (the FC) can consume it directly as
the stationary operand; the natural-orientation ote output is produced
with PE transposes.  Matmuls run as float32r (TF32-precision, full PE
rate); everything else stays fp32.
"""

import os
import numpy as np

import concourse.bass as bass
import concourse.bacc as bacc
import concourse.mybir as mybir
import concourse.tile as tile
from concourse import masks
from concourse.bass_utils import run_bass_kernel_spmd

N_CORES = 8
B, S, H, T = 32, 512, 768, 256
B_LOC = B // N_CORES          # 4 batch rows per core
NS = S // 128                 # 4 s-chunks per row
NJ = B_LOC * NS               # 16 (128,768) x chunks per core
NH = H // 128                 # 6 h-chunks
NB2 = 384                     # stage-2 N tile (two per 768)

F32 = mybir.dt.float32
# float32r = TF32-precision matmul at full PE rate (1 cyc/row for N>=256)
# vs plain fp32 at 4 cyc/row.  Switchable for accuracy fallback.
MM_DT = mybir.dt.float32r if os.environ.get("KERNEL_MM_DT", "f32r") == "f32r" else F32

# tunables (model-searched): engine for each PSUM-evict copy class,
# direct PSUM->DRAM DMA for outputs, psum pool sizes
OPT = {
    "m_copy": "vector",      # meanT psum->sbuf: vector | scalar
    "ote_copy": "vector",    # transpose psum->sbuf: vector | scalar
    "rep_copy": "scalar",    # stage2 psum->sbuf: vector | scalar | dma
    "ote_dma_direct": False, # DMA each transpose psum straight to DRAM
    "p1": 3, "pt": 2, "p2": 3,
    "in_dma": "sync", "out_dma": "scalar",
    "w_after": 3,            # emit fc_w DMAs after this many x2 DMAs
    "bias_mm": True,         # emit the K=1 bias matmuls (False when fc_b==0)
    "x_split_first": True,   # first x2 pair as two 384KB DMAs (earlier PE start)
    "tr_f32r": False,        # PE transposes in f32r (1.5 vs 2 cyc/row)
    "out_split": True,       # output DMAs per row-chunk (1.1us) vs per-b pair
}


def _copy(nc, engine, dst, src_):
    if engine == "scalar":
        nc.scalar.copy(dst, src_)
    else:
        nc.vector.tensor_copy(dst, src_)


def _f32(ap):
    # view an MM_DT tile as plain fp32 (exact datapath, e.g. PE transpose)
    return ap.bitcast(F32) if MM_DT != F32 else ap


def _r(ap):
    # view a fp32 DRAM region as MM_DT for a byte-copy DMA into an MM_DT tile
    return ap.bitcast(MM_DT) if MM_DT != F32 else ap


def build_kernel(reps: int = 1, loop: bool = False,
                 bias_mm: bool | None = None) -> bass.Bass:
    if bias_mm is not None:
        OPT["bias_mm"] = bias_mm
    nc = bacc.Bacc("TRN2", target_bir_lowering=False, debug=False,
                   num_devices=N_CORES)

    x_d = nc.dram_tensor("x", (NJ * 128, H), F32, kind="ExternalInput")
    # packed aux: cols 0..15 = seg, 16..31 = inv  (128, 32)
    aux_d = nc.dram_tensor("selaux", (128, 2 * NJ), F32, kind="ExternalInput")
    fcw_d = nc.dram_tensor("fcwT", (H, H), F32, kind="ExternalInput")
    # packed bias row: [0:H]=fc_b, [H:H+128]=ones
    bias_d = nc.dram_tensor("biasaux", (1, H + 128), F32, kind="ExternalInput")
    identr_d = nc.dram_tensor("identr", (128, 128), F32, kind="ExternalInput")
    rep_d = nc.dram_tensor("rep", (B_LOC * T, H), F32, kind="ExternalOutput")
    ote_d = nc.dram_tensor("ote", (B_LOC * T, H), F32, kind="ExternalOutput")

    # paired-row-chunk views for 768 KB DMAs: [j0][p, q, h] = t[(2*j0+q)*128+p, h]
    x_v = x_d.rearrange("(a q p) h -> a p q h", q=2, p=128)
    rep_v = rep_d.rearrange("(a q p) h -> a p q h", q=2, p=128)
    ote_v = ote_d.rearrange("(a q p) h -> a p q h", q=2, p=128)

    with tile.TileContext(nc) as tc:
        with (
            tc.tile_pool(name="const", bufs=1) as cpool,
            tc.tile_pool(name="xp", bufs=1) as xpool,
            tc.tile_pool(name="selp", bufs=1) as selpool,
            tc.tile_pool(name="mp", bufs=1) as mpool,
            tc.tile_pool(name="wp", bufs=1) as wpool,
            tc.tile_pool(name="ob", bufs=2) as opool,
            tc.tile_pool(name="p1", bufs=OPT["p1"], space="PSUM") as p1pool,
            tc.tile_pool(name="pt", bufs=OPT["pt"], space="PSUM") as ptpool,
            tc.tile_pool(name="p2", bufs=OPT["p2"], space="PSUM") as p2pool,
        ):
            # one-time constants
            iota_t = cpool.tile([128, T], F32, tag="iota")
            nc.gpsimd.iota(iota_t[:], pattern=[[1, T]], base=0,
                           channel_multiplier=0,
                           allow_small_or_imprecise_dtypes=True)
            if OPT["tr_f32r"] and MM_DT != F32:
                ident = cpool.tile([128, 128], MM_DT, tag="ident")
                nc.sync.dma_start(ident[:], _r(identr_d[:]))
                _tr = lambda ap: ap
                TR_DT = MM_DT
            else:
                ident = cpool.tile([128, 128], F32, tag="ident")
                masks.make_identity(nc, ident[:])
                _tr = _f32
                TR_DT = F32

            def emit_rep():
                aux_sb = cpool.tile([128, 2 * NJ], F32, tag="aux")
                bias_sb = cpool.tile([1, H + 128], MM_DT, tag="bias")
                idma = getattr(nc, OPT["in_dma"])
                idma.dma_start(aux_sb[:], aux_d[:])
                idma.dma_start(bias_sb[:], _r(bias_d[:]))
                fcb_sb = bias_sb[:1, 0:H]
                ones = bias_sb[:1, H:H + 128]

                w_sb, x2_sb = [], []

                def emit_w():
                    for k in range(NH):
                        w = wpool.tile([128, H], MM_DT, tag=f"w{k}")
                        idma.dma_start(w[:],
                                       _r(fcw_d[k * 128:(k + 1) * 128, :]))
                        w_sb.append(w)

                for j0 in range(NJ // 2):
                    if j0 == OPT["w_after"]:
                        emit_w()
                    x2 = xpool.tile([128, 2 * H], MM_DT, tag=f"x{j0}")
                    if j0 == 0 and OPT["x_split_first"]:
                        for q in range(2):
                            idma.dma_start(
                                x2[:, q * H:(q + 1) * H],
                                _r(x_d[q * 128:(q + 1) * 128, :]))
                    else:
                        idma.dma_start(
                            x2[:].rearrange("p (q h) -> p q h", q=2),
                            _r(x_v[j0]))
                    x2_sb.append(x2)
                if OPT["w_after"] >= NJ // 2:
                    emit_w()

                def x_chunk(j, mh):
                    # (128,128) stationary slice of wp-token chunk j, h-chunk mh
                    q, j0 = j % 2, j // 2
                    o = q * H + mh * 128
                    return x2_sb[j0][:, o:o + 128]

                sel_sb = []
                for j in range(NJ):
                    sel = selpool.tile([128, T], MM_DT, tag=f"s{j}")
                    # Sel^T chunk: (s==seg member of segment t) * 1/count
                    nc.vector.tensor_scalar(
                        sel[:], iota_t[:],
                        aux_sb[:, j:j + 1], aux_sb[:, NJ + j:NJ + j + 1],
                        op0=mybir.AluOpType.is_equal,
                        op1=mybir.AluOpType.mult)
                    sel_sb.append(sel)

                for b in range(B_LOC):
                    # stage 1: meanT[b] (768, 256) = x[b].T @ SelT[b]
                    mb = []
                    for mh in range(NH):
                        m = mpool.tile([128, T], MM_DT, tag=f"m{b}_{mh}")
                        ps = p1pool.tile([128, T], F32, tag="ps1")
                        for ks in range(NS):
                            j = b * NS + ks
                            nc.tensor.matmul(
                                ps[:],
                                x_chunk(j, mh),
                                sel_sb[j][:],
                                start=(ks == 0), stop=(ks == NS - 1))
                        _copy(nc, OPT["m_copy"], m[:], ps[:])
                        mb.append(m)

                    # ote rows of b: transpose meanT chunks to natural layout
                    odma = getattr(nc, OPT["out_dma"])
                    if OPT["ote_dma_direct"]:
                        for tq in range(2):
                            for mh in range(NH):
                                pt = ptpool.tile([128, 128], TR_DT, tag="pst")
                                nc.tensor.transpose(
                                    pt[:],
                                    _tr(mb[mh][:, tq * 128:(tq + 1) * 128]),
                                    ident[:])
                                r0 = (b * 2 + tq) * 128
                                odma.dma_start(
                                    ote_d[r0:r0 + 128,
                                          mh * 128:(mh + 1) * 128],
                                    _r(pt[:]) if TR_DT != F32 else pt[:])
                    else:
                        osb = opool.tile([128, 2 * H], F32, tag="osb")
                        for tq in range(2):
                            for mh in range(NH):
                                pt = ptpool.tile([128, 128], TR_DT, tag="pst")
                                nc.tensor.transpose(
                                    pt[:],
                                    _tr(mb[mh][:, tq * 128:(tq + 1) * 128]),
                                    ident[:])
                                _copy(nc, OPT["ote_copy"],
                                      osb[:, tq * H + mh * 128:
                                          tq * H + (mh + 1) * 128], pt[:])
                        if OPT["out_split"]:
                            for tq in range(2):
                                r0 = (b * 2 + tq) * 128
                                odma.dma_start(ote_d[r0:r0 + 128, :],
                                               osb[:, tq * H:(tq + 1) * H])
                        else:
                            odma.dma_start(
                                ote_v[b],
                                osb[:].rearrange("p (q h) -> p q h", q=2))

                    # stage 2: rep rows of b = meanT.T @ fc_w.T + fc_b
                    rsb = (None if OPT["rep_copy"] == "dma"
                           else opool.tile([128, 2 * H], F32, tag="rsb"))
                    for tq in range(2):
                        for nh in range(2):
                            ps2 = p2pool.tile([128, NB2], F32, tag="ps2")
                            nsl = slice(nh * NB2, (nh + 1) * NB2)
                            for kh in range(NH):
                                nc.tensor.matmul(
                                    ps2[:],
                                    mb[kh][:, tq * 128:(tq + 1) * 128],
                                    w_sb[kh][:, nsl],
                                    start=(kh == 0),
                                    stop=(not OPT["bias_mm"]
                                          and kh == NH - 1))
                            if OPT["bias_mm"]:
                                nc.tensor.matmul(
                                    ps2[:], ones[:1, :], fcb_sb[:1, nsl],
                                    start=False, stop=True)
                            if OPT["rep_copy"] == "dma":
                                r0 = (b * 2 + tq) * 128
                                odma.dma_start(
                                    rep_d[r0:r0 + 128, nsl], ps2[:])
                            else:
                                _copy(nc, OPT["rep_copy"],
                                      rsb[:, tq * H + nh * NB2:
                                          tq * H + (nh + 1) * NB2], ps2[:])
                    if OPT["rep_copy"] != "dma":
                        if OPT["out_split"]:
                            for tq in range(2):
                                r0 = (b * 2 + tq) * 128
                                odma.dma_start(rep_d[r0:r0 + 128, :],
                                               rsb[:, tq * H:(tq + 1) * H])
                        else:
                            odma.dma_start(
                                rep_v[b],
                                rsb[:].rearrange("p (q h) -> p q h", q=2))

            if loop and reps > 1:
                with tc.For_i(0, reps, 1,
                              hint_engines=(mybir.EngineType.PE,)):
                    emit_rep()
            else:
                for _ in range(reps):
                    emit_rep()

    nc.compile()
    return nc


def _host_prep(all_encoder_layers, token_map, fc_w, fc_b, layer_index):
    """Slice the chosen layer and build per-core input maps."""
    layer = int(np.asarray(layer_index))
    x_full = np.ascontiguousarray(
        np.asarray(all_encoder_layers)[layer], dtype=np.float32)  # (B, S, H)
    tm = np.asarray(token_map).astype(np.int64)                   # (B, T+1)

    pos = np.arange(S)
    seg = np.empty((B, S), dtype=np.int64)
    for b in range(B):
        seg[b] = np.searchsorted(tm[b], pos, side="right") - 1
    valid = pos[None, :] < tm[:, -1:]
    seg = np.where(valid, np.clip(seg, 0, T - 1), T)              # (B, S)
    counts = (tm[:, 1:] - tm[:, :-1]).astype(np.float32)          # (B, T)
    inv = np.zeros((B, S), dtype=np.float32)
    bb = np.arange(B)[:, None]
    iv = seg < T
    inv[iv] = (np.float32(1.0) /
               counts[np.broadcast_to(bb, seg.shape)[iv], seg[iv]])

    fcwT = np.ascontiguousarray(np.asarray(fc_w, dtype=np.float32).T)
    fcb = np.asarray(fc_b, dtype=np.float32).reshape(1, H)

    in_maps = []
    for c in range(N_CORES):
        bs = slice(c * B_LOC, (c + 1) * B_LOC)
        # (B_LOC, S) -> (128, NJ) with column j = b*NS + chunk
        seg_t = seg[bs].reshape(NJ, 128).T.astype(np.float32)
        inv_t = inv[bs].reshape(NJ, 128).T
        aux = np.ascontiguousarray(
            np.concatenate([seg_t, inv_t], axis=1))          # (128, 2*NJ)
        bias_aux = np.ascontiguousarray(np.concatenate(
            [fcb, np.ones((1, 128), np.float32)], axis=1))   # (1, H+128)
        in_maps.append({
            "x": np.ascontiguousarray(x_full[bs].reshape(NJ * 128, H)),
            "selaux": aux,
            "biasaux": bias_aux,
            "fcwT": fcwT,
            "identr": np.eye(128, dtype=np.float32),
        })
    return in_maps


class CachedRunner:
    """Jit/compile/load the bass program once; later calls are pure executes."""

    def __init__(self, nc, donate: bool = True):
        import jax
        from jax.sharding import Mesh, PartitionSpec
        from jax.experimental.shard_map import shard_map
        from concourse import bass2jax

        bass2jax.install_neuronx_cc_hook()
        self.nc = nc
        in_names, out_names, out_avals = [], [], []
        pname = nc.partition_id_tensor.name if nc.partition_id_tensor else None
        for alloc in nc.m.functions[0].allocations:
            if not isinstance(alloc, mybir.MemoryLocationSet):
                continue
            name = alloc.memorylocations[0].name
            if alloc.kind == "ExternalInput":
                if name != pname:
                    in_names.append(name)
            elif alloc.kind == "ExternalOutput":
                shape = tuple(alloc.tensor_shape)
                dtype = mybir.dt.np(alloc.dtype)
                out_names.append(name)
                out_avals.append(jax.core.ShapedArray(shape, dtype))
        self.in_names = list(in_names)
        self.out_names = out_names
        self.out_avals = out_avals
        n_params = len(in_names)
        n_outs = len(out_names)
        all_in_names = list(in_names) + list(out_names)
        if pname is not None:
            all_in_names.append(pname)
        donate_idx = tuple(range(n_params, n_params + n_outs)) if donate else ()

        def _body(*args):
            operands = list(args)
            if pname is not None:
                operands.append(bass2jax.partition_id_tensor())
            outs = bass2jax._bass_exec_p.bind(
                *operands,
                out_avals=tuple(out_avals),
                in_names=tuple(all_in_names),
                out_names=tuple(out_names),
                lowering_input_output_aliases=(),
                sim_require_finite=True,
                sim_require_nnan=True,
                nc=nc,
            )
            return tuple(outs)

        devices = jax.devices()[:N_CORES]
        mesh = Mesh(np.asarray(devices), ("core",))
        in_specs = (PartitionSpec("core"),) * (n_params + n_outs)
        out_specs = (PartitionSpec("core"),) * n_outs
        self.mesh = mesh
        self.sharding = jax.sharding.NamedSharding(mesh, PartitionSpec("core"))
        self.sharded = jax.jit(
            shard_map(_body, mesh=mesh, in_specs=in_specs,
                      out_specs=out_specs, check_rep=False),
            donate_argnums=donate_idx, keep_unused=True)
        self._dev_args = None

    def __call__(self, in_maps):
        concat_in = [
            np.concatenate([np.asarray(in_maps[c][n]) for c in range(N_CORES)], 0)
            for n in self.in_names]
        concat_zeros = [
            np.zeros((N_CORES * a.shape[0], *a.shape[1:]), a.dtype)
            for a in self.out_avals]
        out = self.sharded(*concat_in, *concat_zeros)
        return out  # list of jax arrays, concatenated over cores on axis 0

    def prepare(self, in_maps):
        """device_put all arguments once (requires donate=False runner)."""
        import jax
        concat_in = [
            np.concatenate([np.asarray(in_maps[c][n]) for c in range(N_CORES)], 0)
            for n in self.in_names]
        concat_zeros = [
            np.zeros((N_CORES * a.shape[0], *a.shape[1:]), a.dtype)
            for a in self.out_avals]
        self._dev_args = [jax.device_put(a, self.sharding)
                          for a in concat_in + concat_zeros]
        jax.block_until_ready(self._dev_args)

    def run_prepared(self):
        return self.sharded(*self._dev_args)

    def to_maps(self, out):
        return [
            {n: np.asarray(out[i]).reshape(N_CORES, *self.out_avals[i].shape)[c]
             for i, n in enumerate(self.out_names)}
            for c in range(N_CORES)]


_RUNNER_CACHE: dict = {}


def get_runner(reps: int = 1, loop: bool = False, donate: bool = True,
               bias_mm: bool = True) -> CachedRunner:
    key = (reps, loop, donate, bias_mm)
    if key not in _RUNNER_CACHE:
        _RUNNER_CACHE[key] = CachedRunner(
            build_kernel(reps, loop, bias_mm=bias_mm), donate)
    return _RUNNER_CACHE[key]


def kernel(all_encoder_layers, input_mask, token_map, fc_w, fc_b, layer_index):
    in_maps = _host_prep(all_encoder_layers, token_map, fc_w, fc_b, layer_index)
    bias_mm = bool(np.any(np.asarray(fc_b)))
    runner = get_runner(1, bias_mm=bias_mm)
    out = runner(in_maps)
    idx = {n: i for i, n in enumerate(runner.out_names)}
    rep = np.asarray(out[idx["rep"]])
    ote = np.asarray(out[idx["ote"]])
    return rep.astype(np.float32), ote.astype(np.float32)
